# revision 2
# baseline (speedup 1.0000x reference)
"""Trainium2 Bass kernel for nn_MoEFeedForward (top-2 MoE, E=8, D=1024, H=4096).

Strategy: host-side routing (gate + top-2 + softmax + dispatch), device does the
expert FFN GEMMs. Expert-parallel with load balancing: each core runs S uniform
"segments" of capacity CAP tokens; heavy experts are split across multiple
segments/cores (routing with seed-0 inputs is highly imbalanced). All 8 cores run
one SPMD program; per-core inputs carry the assigned tokens + expert weights.

Device per segment:
  H1T[h, t] = gelu_tanh( W1[:, h]^T X[:, t] + b1[h] )  (GEMM1, bf16, f32 psum)
  YT[d, t]  = W2[:, d]^T H1T[:, t] + b2[d]             (GEMM2)
Host combines: out[tok] += w_slot * YT[:, slot].T ; usage = bincount(idx).
"""

import functools
import os
import sys
import types

import numpy as np
import ml_dtypes

B, L, D, H, E, TOPK = 2, 2048, 1024, 4096, 8, 2
T = B * L
NCORES = 8
P = 128
KD = D // P   # 8 contraction tiles for GEMM1 / output tiles for GEMM2
MH = H // P   # 32 output tiles for GEMM1 / contraction tiles for GEMM2

_BF16 = ml_dtypes.bfloat16

LAST_RESULT = None  # BassKernelResults of the most recent device run


def _install_axon_profile_shim():
    """Make run_bass_kernel_spmd(trace=True) work in this slim container:
    provide antenv.axon_hooks backed by the ctypes NTFF hook, and disable
    cloud artifact uploads."""
    if "antenv.axon_hooks" in sys.modules:
        return
    try:
        from trn_agent_boot.trn_boot import _ntff_profile_via_ctypes

        hook = _ntff_profile_via_ctypes("/opt/axon/libaxon_pjrt.so")
    except Exception:
        hook = None
    mod = types.ModuleType("antenv.axon_hooks")
    mod.get_axon_ntff_profile_hook = lambda: hook
    mod.set_axon_ntff_profile_hook = lambda h: None
    sys.modules["antenv.axon_hooks"] = mod
    try:
        import concourse.bass_utils as bu

        bu.upload_artifacts = lambda tmpdir: f"file://{tmpdir}"
    except Exception:
        pass


def _gate(hidden_states, style_emb, Wg):
    """Replicate the reference gate in float64 (min 2nd/3rd logit gap for these
    inputs is ~4e-5, far above f32-vs-f64 noise, so top-k indices are exact)."""
    x = hidden_states.astype(np.float64).reshape(T, D)
    s = np.repeat(style_emb.astype(np.float64), L, axis=0)  # [T, D]
    logits = (x + s) @ Wg.astype(np.float64).T  # [T, E]
    i0 = np.argmax(logits, axis=1)
    tmp = logits.copy()
    tmp[np.arange(T), i0] = -np.inf
    i1 = np.argmax(tmp, axis=1)
    v0 = logits[np.arange(T), i0]
    v1 = logits[np.arange(T), i1]
    e1 = np.exp(v1 - v0)
    z = 1.0 + e1
    w = np.stack([1.0 / z, e1 / z], axis=1).astype(np.float32)  # [T, 2]
    idx = np.stack([i0, i1], axis=1).astype(np.int32)  # [T, 2]
    return idx, w


def _choose_cap(counts):
    """Pick (S, CAP): each core runs S segments of CAP tokens. Minimize modeled
    per-core time: S * max(compute cycles, weight-DMA cycles) + per-seg overhead."""
    DMA_CYC = 112_000  # 16.8MB weights / 360GB/s in 2.4GHz cycles
    SEG_OVERHEAD = 7_000
    best = None
    for cap in range(224, 513):
        nseg = sum((int(c) + cap - 1) // cap for c in counts if c > 0)
        S = max(1, (nseg + NCORES - 1) // NCORES)
        cost = S * max(512 * cap, DMA_CYC) + S * SEG_OVERHEAD
        key = (cost, S, -cap)
        if best is None or key < best[0]:
            best = (key, S, cap)
    return best[1], best[2]


@functools.lru_cache(maxsize=4)
def _build_program(S, CAP):
    import concourse.tile as tile
    import concourse.mybir as mybir
    from concourse import bacc

    dt = mybir.dt
    GELU = mybir.ActivationFunctionType.Gelu_apprx_tanh
    IDENT = mybir.ActivationFunctionType.Identity

    nc = bacc.Bacc("TRN2", target_bir_lowering=False, debug=False,
                   num_devices=NCORES)
    xt = nc.dram_tensor("xt", [S, P, KD, CAP], dt.bfloat16, kind="ExternalInput")
    w1 = nc.dram_tensor("w1", [S, 4, P, KD, 1024], dt.bfloat16, kind="ExternalInput")
    w2 = nc.dram_tensor("w2", [S, MH, P, D], dt.bfloat16, kind="ExternalInput")
    b1t = nc.dram_tensor("b1t", [S, P, MH], dt.float32, kind="ExternalInput")
    b2t = nc.dram_tensor("b2t", [S, P, KD], dt.float32, kind="ExternalInput")
    yt = nc.dram_tensor("yt", [S, P, KD, CAP], dt.float32, kind="ExternalOutput")

    with tile.TileContext(nc) as tc:
        with tc.tile_pool(name="xtp", bufs=2) as xtp, \
             tc.tile_pool(name="w1p", bufs=3) as w1p, \
             tc.tile_pool(name="w2p", bufs=6) as w2p, \
             tc.tile_pool(name="h1p", bufs=2) as h1p, \
             tc.tile_pool(name="ytp", bufs=2) as ytp, \
             tc.tile_pool(name="bp", bufs=2) as bp, \
             tc.tile_pool(name="ps", bufs=8, space="PSUM") as ps:
            for s in range(S):
                xs = xtp.tile([P, KD, CAP], dt.bfloat16, tag="xt")
                nc.sync.dma_start(xs[:], xt.ap()[s])
                b1s = bp.tile([P, MH], dt.float32, tag="b1")
                nc.sync.dma_start(b1s[:], b1t.ap()[s])
                b2s = bp.tile([P, KD], dt.float32, tag="b2")
                nc.sync.dma_start(b2s[:], b2t.ap()[s])
                h1 = h1p.tile([P, MH, CAP], dt.bfloat16, tag="h1")

                # GEMM1: H1T[mh] = gelu(sum_kd W1[kd,mh]^T @ X[kd] + b1)
                for mhg in range(4):
                    w1s = w1p.tile([P, KD, 1024], dt.bfloat16, tag="w1")
                    nc.sync.dma_start(w1s[:], w1.ap()[s, mhg])
                    for half in range(2):
                        pts = [ps.tile([P, CAP], dt.float32, tag="ps",
                                       name=f"ps_g1_{s}_{mhg}_{half}_{i}")
                               for i in range(4)]
                        for kd in range(KD):
                            for i in range(4):
                                c0 = (half * 4 + i) * P
                                nc.tensor.matmul(
                                    pts[i][:], w1s[:, kd, c0:c0 + P],
                                    xs[:, kd, :],
                                    start=(kd == 0), stop=(kd == KD - 1))
                        for i in range(4):
                            mh = mhg * 8 + half * 4 + i
                            nc.scalar.activation(
                                h1[:, mh, :], pts[i][:], GELU,
                                bias=b1s[:, mh:mh + 1])

                # GEMM2: YT[md] = sum_kh W2[kh,md]^T @ H1T[kh] + b2
                ys = ytp.tile([P, KD, CAP], dt.float32, tag="yt")
                pts2 = [ps.tile([P, CAP], dt.float32, tag="ps",
                                name=f"ps_g2_{s}_{i}")
                        for i in range(KD)]
                for kh in range(MH):
                    w2s = w2p.tile([P, D], dt.bfloat16, tag="w2")
                    nc.sync.dma_start(w2s[:], w2.ap()[s, kh])
                    for md in range(KD):
                        nc.tensor.matmul(
                            pts2[md][:], w2s[:, md * P:(md + 1) * P],
                            h1[:, kh, :],
                            start=(kh == 0), stop=(kh == MH - 1))
                for md in range(KD):
                    nc.scalar.activation(
                        ys[:, md, :], pts2[md][:], IDENT,
                        bias=b2s[:, md:md + 1])
                nc.sync.dma_start(yt.ap()[s], ys[:])

    nc.compile()
    return nc


@functools.lru_cache(maxsize=2)
def _expert_layouts_cached(key):
    raise RuntimeError  # placeholder; real caching done in kernel() scope


def _prep_expert(W1e, b1e, W2e, b2e):
    """Device layouts for one expert (host-side, cheap)."""
    w1l = np.ascontiguousarray(
        W1e.astype(_BF16).reshape(KD, P, 4, 1024).transpose(2, 1, 0, 3))
    w2l = np.ascontiguousarray(W2e.astype(_BF16).reshape(MH, P, D))
    b1l = np.ascontiguousarray(b1e.astype(np.float32).reshape(MH, P).T)
    b2l = np.ascontiguousarray(b2e.astype(np.float32).reshape(KD, P).T)
    return w1l, w2l, b1l, b2l


def kernel(hidden_states, style_emb, Wg, W1, b1, W2, b2):
    global LAST_RESULT
    _install_axon_profile_shim()
    from concourse.bass_utils import run_bass_kernel_spmd

    hidden_states = np.asarray(hidden_states, dtype=np.float32)
    style_emb = np.asarray(style_emb, dtype=np.float32)
    Wg = np.asarray(Wg, dtype=np.float32)
    W1 = np.asarray(W1, dtype=np.float32)
    b1 = np.asarray(b1, dtype=np.float32)
    W2 = np.asarray(W2, dtype=np.float32)
    b2 = np.asarray(b2, dtype=np.float32)

    idx, wts = _gate(hidden_states, style_emb, Wg)
    counts = np.bincount(idx.ravel(), minlength=E)
    usage = counts.astype(np.float32)
    S, CAP = _choose_cap(counts)

    nc = _build_program(S, CAP)

    xf = hidden_states.reshape(T, D)

    # Split each expert's token list into chunks of <= CAP slots.
    chunks = []  # (expert, tok_ids, w_sel)
    for e in range(E):
        tok, kk = np.nonzero(idx == e)
        if len(tok) == 0:
            continue
        we = wts[tok, kk]
        for off in range(0, len(tok), CAP):
            chunks.append((e, tok[off:off + CAP], we[off:off + CAP]))
    assert len(chunks) <= NCORES * S, (len(chunks), NCORES, S)

    expert_layouts = {}
    in_maps = [{
        "xt": np.zeros([S, P, KD, CAP], _BF16),
        "w1": np.zeros([S, 4, P, KD, 1024], _BF16),
        "w2": np.zeros([S, MH, P, D], _BF16),
        "b1t": np.zeros([S, P, MH], np.float32),
        "b2t": np.zeros([S, P, KD], np.float32),
    } for _ in range(NCORES)]

    placement = []  # (core, seg, expert, tok_ids, w_sel)
    for i, (e, tok, we) in enumerate(chunks):
        core, seg = i % NCORES, i // NCORES
        if e not in expert_layouts:
            expert_layouts[e] = _prep_expert(W1[e], b1[e], W2[e], b2[e])
        w1l, w2l, b1l, b2l = expert_layouts[e]
        m = in_maps[core]
        m["w1"][seg] = w1l
        m["w2"][seg] = w2l
        m["b1t"][seg] = b1l
        m["b2t"][seg] = b2l
        n = len(tok)
        xc = xf[tok].astype(_BF16)  # [n, D]
        m["xt"][seg, :, :, :n] = xc.T.reshape(KD, P, n).transpose(1, 0, 2)
        placement.append((core, seg, e, tok, we))

    trace = bool(int(os.environ.get("KERNEL_TRACE", "0")))
    res = run_bass_kernel_spmd(
        nc, in_maps, core_ids=list(range(NCORES)), trace=trace,
        tmpdir=os.environ.get("KERNEL_TMPDIR"))
    LAST_RESULT = res

    out = np.zeros((T, D), dtype=np.float32)
    for core, seg, e, tok, we in placement:
        n = len(tok)
        ytc = res.results[core]["yt"][seg]  # [P, KD, CAP] f32
        y = ytc[:, :, :n].transpose(2, 1, 0).reshape(n, D)  # [n, D]
        out[tok] += we[:, None] * y
    return out.reshape(B, L, D), usage
